# revision 1
# baseline (speedup 1.0000x reference)
# Trainium2 Bass kernel for nn_CorrelationModule (RAFT-style windowed correlation + DAP).
#
# Math: for each pixel p=(h,w) with coords (cx, cy):
#   corr[i,j,p] = (1/16) * sum_c bilinear(f2, cx+i-4, cy+j-4)[c] * f1[c,p]
#   out[o,p]    = sum_d w_dap[o,d] * corr[d,p],  d = i*9+j
# Since the bilinear fractional weights are shared across all 81 window offsets,
#   corr[i,j,p] = sum_{a,b in {0,1}} wx_a wy_b G[p, j+b, i+a]
# where G[p,v,u] = f1[:,p] . f2[:, y0+v-4, x0+u-4]  (zero when out of bounds),
# x0=floor(cx), y0=floor(cy). G is computed with dense matmuls against whole
# f2 rows: pixels are bucketed by y0 (8 buckets of 576, sorted), each bucket
# only needs an 18-row window of f2. The row-dot volume goes to DRAM; a single
# indirect-DMA index per pixel gathers its contiguous 874-float span covering
# the whole 10x10 window at the pixel's x-offset; the bilinear combine, a PE
# transpose, and the 81x81 DAP matmul finish the job on-chip.
#
# Sharding: data-parallel over pixels, bucketed by floor(cy) so each core's
# f2 slice is 18 rows (4-row halo each side). Host does the permutation as
# part of sharding and inverts it on gather.

import numpy as np

import concourse.bass as bass
import concourse.mybir as mybir
import concourse.tile as tile
from concourse import bacc
from concourse.bass_utils import run_bass_kernel_spmd
from concourse.masks import make_identity

f32 = mybir.dt.float32
i32 = mybir.dt.int32

# Problem constants (hardcoded per contest rules).
B, C, H, W = 1, 256, 48, 96
R = 4
D = 2 * R + 1           # 9
HW = H * W              # 4608
NCORES = 8
PIX = HW // NCORES      # 576 pixels per core
NT = 5                  # M-tiles of 128 (640 = padded pixel count)
PPAD = NT * 128         # 640
WINROWS = 18            # f2 row window per core (span<=9 + 4-halo each side)
W18 = WINROWS * W       # 1728
STRIDE = 1792           # vol row stride (pad so gathers never leave own row)
GAT = 9 * W + 10        # 874 contiguous elems per pixel gather
VROWS = 128             # per-M-tile vol tensor rows
HEAD = 4                # leading zero cols in f2sb; makes gather offsets >= 0
CHUNKS = [(0, 512), (512, 512), (1024, 512), (1536, 256)]  # matmul N chunks


def _alu(name):
    return getattr(mybir.AluOpType, name)


def _floor_frac(nc, pool, x, P, NCOL, pfx):
    """floor and frac of nonneg f32 tile x [P, NCOL] (conversion-mode agnostic)."""
    xi = pool.tile([P, NCOL], i32, name=pfx + "xi")
    nc.vector.tensor_copy(out=xi[:], in_=x)
    xf = pool.tile([P, NCOL], f32, name=pfx + "xf")
    nc.vector.tensor_copy(out=xf[:], in_=xi[:])
    d = pool.tile([P, NCOL], f32, name=pfx + "d")
    nc.vector.tensor_tensor(out=d[:], in0=x, in1=xf[:], op=_alu("subtract"))
    negm = pool.tile([P, NCOL], f32, name=pfx + "negm")
    nc.vector.tensor_scalar(out=negm[:], in0=d[:], scalar1=0.0, scalar2=None,
                            op0=_alu("is_lt"))
    fl = pool.tile([P, NCOL], f32, name=pfx + "fl")
    nc.vector.tensor_tensor(out=fl[:], in0=xf[:], in1=negm[:], op=_alu("subtract"))
    fr = pool.tile([P, NCOL], f32, name=pfx + "fr")
    nc.vector.tensor_tensor(out=fr[:], in0=x, in1=fl[:], op=_alu("subtract"))
    return fl, fr


def build_program(mm_dtype=f32, fake_gather=False, vol_bf16=False, wbufs=2, vol_dma_eng="sync"):
    """Build the (SPMD, per-core) bass program. All shapes static.

    fake_gather=True (analysis only): replace the indirect gather with a
    static strided DMA of identical descriptor structure (128 descriptors x
    3496 B from the same DRAM tensor) so the cost-model timeline reflects the
    real HW descriptor count. Produces wrong data - never use for results.

    vol_bf16=True: stage the row-dot volume in bf16 (halves the DRAM round
    trip; ~0.4% relative rounding on the pre-combine dots).
    """
    vdt = mybir.dt.bfloat16 if vol_bf16 else f32
    nc = bacc.Bacc()
    P = 128

    # matmul input handling by dtype:
    #  float32  - plain loads, 4 cyc/row
    #  float32r - declare params+tiles as float32r (same bits as f32), 1 cyc/row
    #  bfloat16 - SWDGE casts f32->bf16 during the load, 1 cyc/row
    mdt = mm_dtype
    pdt = mdt if mdt == mybir.dt.float32r else f32
    f1g = nc.declare_dram_parameter("f1g", [C, PPAD], pdt, isOutput=False)
    f2win = nc.declare_dram_parameter("f2win", [C, W18], pdt, isOutput=False)
    coords_t = nc.declare_dram_parameter("coords_t", [PPAD, 4], f32, isOutput=False)
    w_dapT = nc.declare_dram_parameter("w_dapT", [D * D, D * D], f32, isOutput=False)
    out_p = nc.declare_dram_parameter("out", [D * D, PPAD], f32, isOutput=True)

    vols = [nc.dram_tensor(f"vol{t}", [VROWS * STRIDE], vdt) for t in range(NT)]

    with tile.TileContext(nc) as tc:
        with tc.tile_pool(name="const", bufs=1) as cpool, \
             tc.tile_pool(name="work", bufs=wbufs) as wpool, \
             tc.tile_pool(name="mmps", bufs=6, space="PSUM") as mmpool, \
             tc.tile_pool(name="tps", bufs=1, space="PSUM") as tpool, \
             tc.tile_pool(name="ops", bufs=1, space="PSUM") as opool:

            # ---------------- loads ----------------
            mm_cast = (mdt == mybir.dt.bfloat16)
            dma_eng = nc.gpsimd if mm_cast else nc.sync
            f1sb = []
            for k in range(2):
                t_ = cpool.tile([P, PPAD], mdt, name=f"f1sb{k}")
                dma_eng.dma_start(out=t_[:], in_=f1g[k * P:(k + 1) * P, :])
                f1sb.append(t_)
            f2sb = []
            for k in range(2):
                # layout: [0:HEAD)=0, [HEAD:HEAD+W18)=f2 rows, rest 0
                t_ = cpool.tile([P, STRIDE], mdt, name=f"f2sb{k}")
                # memset doesn't accept float32r; zero through a f32 view
                zv0 = t_[:, 0:HEAD]
                zv1 = t_[:, HEAD + W18:STRIDE]
                if mdt == mybir.dt.float32r:
                    zv0, zv1 = zv0.bitcast(f32), zv1.bitcast(f32)
                nc.vector.memset(zv0, 0.0)
                nc.vector.memset(zv1, 0.0)
                dma_eng.dma_start(out=t_[:, HEAD:HEAD + W18], in_=f2win[k * P:(k + 1) * P, :])
                f2sb.append(t_)
            wd = cpool.tile([D * D, D * D], f32, name="wd")
            nc.sync.dma_start(out=wd[:], in_=w_dapT[:, :])
            ident = cpool.tile([P, P], f32, name="ident")
            make_identity(nc, ident[:])
            ct = cpool.tile([P, NT, 4], f32, name="ct")
            nc.sync.dma_start(out=ct[:], in_=coords_t.rearrange("(t l) c -> l t c", l=P))

            # ---------------- per-pixel prep (once) ----------------
            cx = cpool.tile([P, NT], f32, name="cx")
            nc.vector.tensor_copy(out=cx[:], in_=ct[:, :, 0])
            cy = cpool.tile([P, NT], f32, name="cy")
            nc.vector.tensor_copy(out=cy[:], in_=ct[:, :, 1])
            cyadj = cpool.tile([P, NT], f32, name="cyadj")
            nc.vector.tensor_copy(out=cyadj[:], in_=ct[:, :, 2])

            x0f, fx = _floor_frac(nc, cpool, cx[:], P, NT, 'x_')
            y0f, fy = _floor_frac(nc, cpool, cy[:], P, NT, 'y_')
            # floor(cyadj) = cyadj - fy (rowstart integer shift keeps frac)
            y0adjf = cpool.tile([P, NT], f32, name="y0adjf")
            nc.vector.tensor_tensor(out=y0adjf[:], in0=cyadj[:], in1=fy[:],
                                    op=_alu("subtract"))

            xsf = cpool.tile([P, NT], f32, name="xsf")   # x0 - 4
            nc.vector.tensor_scalar(out=xsf[:], in0=x0f[:], scalar1=-4.0,
                                    scalar2=None, op0=_alu("add"))
            ysf = cpool.tile([P, NT], f32, name="ysf")   # y0 - 4 (true rows)
            nc.vector.tensor_scalar(out=ysf[:], in0=y0f[:], scalar1=-4.0,
                                    scalar2=None, op0=_alu("add"))

            # gather offsets: p*STRIDE + (y0adj-4)*96 + (x0-4) + HEAD
            basei = cpool.tile([P, 1], i32, name="basei")
            nc.gpsimd.iota(basei[:], pattern=[[0, 1]], base=HEAD,
                           channel_multiplier=STRIDE)
            basef = cpool.tile([P, 1], f32, name="basef")
            nc.vector.tensor_copy(out=basef[:], in_=basei[:])
            offf = cpool.tile([P, NT], f32, name="offf")
            nc.vector.tensor_scalar(out=offf[:], in0=y0adjf[:], scalar1=96.0,
                                    scalar2=float(-4 * 96 - 4), op0=_alu("mult"),
                                    op1=_alu("add"))
            nc.vector.tensor_tensor(out=offf[:], in0=offf[:], in1=x0f[:], op=_alu("add"))
            nc.vector.tensor_tensor(out=offf[:], in0=offf[:],
                                    in1=basef[:].to_broadcast([P, NT]), op=_alu("add"))
            offi = cpool.tile([P, NT], i32, name="offi")
            nc.vector.tensor_copy(out=offi[:], in_=offf[:])

            # bilinear weights (y weights carry the 1/16 scale)
            wx0 = cpool.tile([P, NT], f32, name="wx0")
            nc.vector.tensor_scalar(out=wx0[:], in0=fx[:], scalar1=-1.0, scalar2=1.0,
                                    op0=_alu("mult"), op1=_alu("add"))
            wy0 = cpool.tile([P, NT], f32, name="wy0")
            nc.vector.tensor_scalar(out=wy0[:], in0=fy[:], scalar1=-0.0625, scalar2=0.0625,
                                    op0=_alu("mult"), op1=_alu("add"))
            wy1 = cpool.tile([P, NT], f32, name="wy1")
            nc.vector.tensor_scalar(out=wy1[:], in0=fy[:], scalar1=0.0625, scalar2=None,
                                    op0=_alu("mult"))

            # iota 0..9 as f32
            iu = cpool.tile([P, 10], i32, name="iu")
            nc.gpsimd.iota(iu[:], pattern=[[1, 10]], base=0, channel_multiplier=0)
            iuf = cpool.tile([P, 10], f32, name="iuf")
            nc.vector.tensor_copy(out=iuf[:], in_=iu[:])

            # validity: vx[p,t,u] = 0 <= x0-4+u <= 95 ; vy similar with 47
            def _valid(base_tile, hi, nm):
                col = cpool.tile([P, NT, 10], f32, name=nm + "c")
                nc.vector.tensor_tensor(
                    out=col[:], in0=base_tile[:].unsqueeze(2).to_broadcast([P, NT, 10]),
                    in1=iuf[:].unsqueeze(1).to_broadcast([P, NT, 10]), op=_alu("add"))
                v0 = cpool.tile([P, NT, 10], f32, name=nm + "0")
                nc.vector.tensor_scalar(out=v0[:], in0=col[:], scalar1=0.0,
                                        scalar2=None, op0=_alu("is_ge"))
                v1 = cpool.tile([P, NT, 10], f32, name=nm + "1")
                nc.vector.tensor_scalar(out=v1[:], in0=col[:], scalar1=float(hi),
                                        scalar2=None, op0=_alu("is_le"))
                v = cpool.tile([P, NT, 10], f32, name=nm)
                nc.vector.tensor_tensor(out=v[:], in0=v0[:], in1=v1[:], op=_alu("mult"))
                return v

            vx = _valid(xsf, W - 1, "vx")
            vy = _valid(ysf, H - 1, "vy")

            # fused weight*validity tables [P, NT, 9]
            def _wtab(v, sl, wsc, nm):
                o = cpool.tile([P, NT, 9], f32, name=nm)
                nc.vector.tensor_tensor(
                    out=o[:], in0=v[:, :, sl],
                    in1=wsc[:].unsqueeze(2).to_broadcast([P, NT, 9]), op=_alu("mult"))
                return o

            X0a = _wtab(vx, slice(0, 9), wx0, "X0a")
            X1a = _wtab(vx, slice(1, 10), fx, "X1a")
            W0a = _wtab(vy, slice(0, 9), wy0, "W0a")
            W1a = _wtab(vy, slice(1, 10), wy1, "W1a")

            outsb_all = cpool.tile([D * D, NT, P], f32, name="outsb_all")

            # ---------------- main loop over M-tiles ----------------
            for t in range(NT):
                # matmul: vol[p, 1792] = f1g_tile^T @ f2win(padded)
                psums = []
                for ci, (n0, w_) in enumerate(CHUNKS):
                    ps = mmpool.tile([P, 512], f32, tag="mmps")
                    for k in range(2):
                        nc.tensor.matmul(out=ps[:, :w_],
                                         lhsT=f1sb[k][:, t * P:(t + 1) * P],
                                         rhs=f2sb[k][:, n0:n0 + w_],
                                         start=(k == 0), stop=(k == 1))
                    psums.append(ps)
                volsb = wpool.tile([P, STRIDE], vdt, tag="volsb")
                for ci, (n0, w_) in enumerate(CHUNKS):
                    eng = nc.vector if ci % 2 == 0 else nc.scalar
                    if eng is nc.scalar:
                        nc.scalar.copy(out=volsb[:, n0:n0 + w_], in_=psums[ci][:, :w_])
                    else:
                        nc.vector.tensor_copy(out=volsb[:, n0:n0 + w_], in_=psums[ci][:, :w_])
                vol2d = vols[t].rearrange("(r s) -> r s", s=STRIDE)
                getattr(nc, vol_dma_eng).dma_start(out=vol2d[:, :], in_=volsb[:])

                # indirect gather: one 874-elem contiguous span per pixel
                g = wpool.tile([P, GAT], vdt, tag="g874")
                if fake_gather:
                    nc.gpsimd.dma_start(out=g[:, :], in_=vol2d[:, :GAT])
                else:
                    volv = vols[t].rearrange("(n one) -> n one", one=1)
                    nc.gpsimd.indirect_dma_start(
                        out=g[:, :], out_offset=None, in_=volv[:, :],
                        in_offset=bass.IndirectOffsetOnAxis(ap=offi[:, t:t + 1], axis=0))

                # Hx: hx[p, i*10+v] = X0[p,i]*E[p,v,i] + X1[p,i]*E[p,v,i+1]
                #   E[p,v,u] = g[p, v*96+u]
                gap = g[:]
                pdim = gap.ap[0]
                E0 = bass.AP(gap.tensor, gap.offset, [pdim, [1, 9], [W, 10]])
                E1 = bass.AP(gap.tensor, gap.offset + 1, [pdim, [1, 9], [W, 10]])
                hx0 = wpool.tile([P, 90], f32, tag="hx0")
                hx1 = wpool.tile([P, 90], f32, tag="hx1")
                hxs = wpool.tile([P, 90], f32, tag="hxs")
                h3 = lambda x: x[:].rearrange("p (i v) -> p i v", i=9)
                nc.vector.tensor_tensor(
                    out=h3(hx0), in0=E0,
                    in1=X0a[:, t, :].unsqueeze(2).to_broadcast([P, 9, 10]), op=_alu("mult"))
                nc.vector.tensor_tensor(
                    out=h3(hx1), in0=E1,
                    in1=X1a[:, t, :].unsqueeze(2).to_broadcast([P, 9, 10]), op=_alu("mult"))
                nc.vector.tensor_tensor(out=hxs[:], in0=hx0[:], in1=hx1[:], op=_alu("add"))

                # Hy: corr[p, i*9+j] = W0[p,j]*hx[p,i*10+j] + W1[p,j]*hx[p,i*10+j+1]
                hxv = h3(hxs)
                c0 = wpool.tile([P, 81], f32, tag="c0")
                c1 = wpool.tile([P, 81], f32, tag="c1")
                corr = wpool.tile([P, 81], f32, tag="corr")
                c3 = lambda x: x[:].rearrange("p (i j) -> p i j", i=9)
                nc.vector.tensor_tensor(
                    out=c3(c0), in0=hxv[:, :, 0:9],
                    in1=W0a[:, t, :].unsqueeze(1).to_broadcast([P, 9, 9]), op=_alu("mult"))
                nc.vector.tensor_tensor(
                    out=c3(c1), in0=hxv[:, :, 1:10],
                    in1=W1a[:, t, :].unsqueeze(1).to_broadcast([P, 9, 9]), op=_alu("mult"))
                nc.vector.tensor_tensor(out=corr[:], in0=c0[:], in1=c1[:], op=_alu("add"))

                # transpose -> [81, 128], then DAP matmul
                psT = tpool.tile([D * D, P], f32, tag="tps")
                nc.tensor.transpose(out=psT[:], in_=corr[:], identity=ident[:])
                corrT = wpool.tile([D * D, P], f32, tag="corrT")
                nc.scalar.copy(out=corrT[:], in_=psT[:])
                psO = opool.tile([D * D, P], f32, tag="ops")
                nc.tensor.matmul(out=psO[:], lhsT=wd[:], rhs=corrT[:],
                                 start=True, stop=True)
                nc.scalar.copy(out=outsb_all[:, t, :], in_=psO[:])
            nc.sync.dma_start(out=out_p[:, :], in_=outsb_all[:].rearrange("o t p -> o (t p)"))

    nc.compile()
    return nc


def shard_inputs(f1, f2, coords):
    """Bucket pixels by floor(cy), split into 8 equal chunks, build per-core arrays."""
    cx = np.asarray(coords[0, 0].reshape(-1), dtype=np.float32)
    cy = np.asarray(coords[0, 1].reshape(-1), dtype=np.float32)
    y0 = np.floor(cy).astype(np.int64)
    perm = np.argsort(y0, kind="stable")
    f1f = np.asarray(f1[0].reshape(C, HW), dtype=np.float32)
    f2f = np.asarray(f2[0], dtype=np.float32)  # [C, H, W]

    in_maps = []
    for c in range(NCORES):
        sel = perm[c * PIX:(c + 1) * PIX]
        ys = y0[sel]
        y_lo, y_hi = int(ys.min()), int(ys.max())
        assert y_hi - y_lo <= 8, f"core {c}: y0 span {y_hi - y_lo + 1} > 9 unsupported"
        rowstart = y_lo - 4
        rows = np.clip(np.arange(rowstart, rowstart + WINROWS), 0, H - 1)
        f2w = np.ascontiguousarray(f2f[:, rows, :].reshape(C, W18))

        f1gc = np.zeros((C, PPAD), dtype=np.float32)
        f1gc[:, :PIX] = f1f[:, sel]
        ctc = np.zeros((PPAD, 4), dtype=np.float32)
        ctc[:PIX, 0] = cx[sel]
        ctc[:PIX, 1] = cy[sel]
        ctc[:PIX, 2] = cy[sel] - rowstart
        # pad pixels: mid-window coords so all indices stay in range
        ctc[PIX:, 0] = 10.3
        ctc[PIX:, 1] = 10.3
        ctc[PIX:, 2] = 8.3
        in_maps.append({"f1g": f1gc, "f2win": f2w, "coords_t": ctc})
    return in_maps, perm


_PROG_CACHE = {}


def _run(f1, f2, coords, w_dap, mm_dtype_name="float32", trace=False):
    mm_dtype = getattr(mybir.dt, mm_dtype_name)
    key = mm_dtype_name
    if key not in _PROG_CACHE:
        _PROG_CACHE[key] = build_program(mm_dtype=mm_dtype)
    nc = _PROG_CACHE[key]

    in_maps, perm = shard_inputs(f1, f2, coords)
    wdT = np.ascontiguousarray(np.asarray(w_dap, dtype=np.float32).T)
    for m in in_maps:
        m["w_dapT"] = wdT

    res = run_bass_kernel_spmd(nc, in_maps, core_ids=list(range(NCORES)), trace=trace)
    sorted_out = np.concatenate(
        [res.results[c]["out"][:, :PIX] for c in range(NCORES)],
        axis=1)  # [81, 4608] in sorted-pixel order
    full = np.empty((D * D, HW), dtype=np.float32)
    full[:, perm] = sorted_out
    out = full.reshape(1, D * D, H, W)
    return out, res


def kernel(f1, f2, coords, w_dap):
    out, _ = _run(f1, f2, coords, w_dap)
    return out



# revision 3
# speedup vs baseline: 10.9397x; 10.9397x over previous
# Trainium2 Bass kernel for nn_CorrelationModule (RAFT-style windowed correlation + DAP).
#
# Math: for each pixel p=(h,w) with coords (cx, cy):
#   corr[i,j,p] = (1/16) * sum_c bilinear(f2, cx+i-4, cy+j-4)[c] * f1[c,p]
#   out[o,p]    = sum_d w_dap[o,d] * corr[d,p],  d = i*9+j
# Since the bilinear fractional weights are shared across all 81 window offsets,
#   corr[i,j,p] = sum_{a,b in {0,1}} wx_a wy_b G[p, j+b, i+a]
# where G[p,v,u] = f1[:,p] . f2[:, y0+v-4, x0+u-4]  (zero when out of bounds),
# x0=floor(cx), y0=floor(cy). G is computed with dense matmuls against whole
# f2 rows: pixels are bucketed by y0 (8 buckets of 576, sorted), each bucket
# only needs an 18-row window of f2. The row-dot volume goes to DRAM; a single
# indirect-DMA index per pixel gathers its contiguous 874-float span covering
# the whole 10x10 window at the pixel's x-offset; the bilinear combine, a PE
# transpose, and the 81x81 DAP matmul finish the job on-chip.
#
# Sharding: data-parallel over pixels, bucketed by floor(cy) so each core's
# f2 slice is 18 rows (4-row halo each side). Host does the permutation as
# part of sharding and inverts it on gather.

import numpy as np

import concourse.bass as bass
import concourse.mybir as mybir
import concourse.tile as tile
from concourse import bacc
from concourse.bass_utils import run_bass_kernel_spmd
from concourse.masks import make_identity

f32 = mybir.dt.float32
i32 = mybir.dt.int32

# Problem constants (hardcoded per contest rules).
B, C, H, W = 1, 256, 48, 96
R = 4
D = 2 * R + 1           # 9
HW = H * W              # 4608
NCORES = 8
PIX = HW // NCORES      # 576 pixels per core
NT = 5                  # M-tiles of 128 (640 = padded pixel count)
PPAD = NT * 128         # 640
WINROWS = 18            # f2 row window per core (span<=9 + 4-halo each side)
W18 = WINROWS * W       # 1728
STRIDE = 1792           # vol row stride (pad so gathers never leave own row)
GAT = 9 * W + 10        # 874 contiguous elems per pixel gather
VROWS = 128             # per-M-tile vol tensor rows
HEAD = 4                # leading zero cols in f2sb; makes gather offsets >= 0
CHUNKS = [(0, 512), (512, 512), (1024, 512), (1536, 256)]  # matmul N chunks


def _alu(name):
    return getattr(mybir.AluOpType, name)


def _floor_frac(nc, pool, x, P, NCOL, pfx):
    """floor and frac of nonneg f32 tile x [P, NCOL] (conversion-mode agnostic)."""
    xi = pool.tile([P, NCOL], i32, name=pfx + "xi")
    nc.vector.tensor_copy(out=xi[:], in_=x)
    xf = pool.tile([P, NCOL], f32, name=pfx + "xf")
    nc.vector.tensor_copy(out=xf[:], in_=xi[:])
    d = pool.tile([P, NCOL], f32, name=pfx + "d")
    nc.vector.tensor_tensor(out=d[:], in0=x, in1=xf[:], op=_alu("subtract"))
    negm = pool.tile([P, NCOL], f32, name=pfx + "negm")
    nc.vector.tensor_scalar(out=negm[:], in0=d[:], scalar1=0.0, scalar2=None,
                            op0=_alu("is_lt"))
    fl = pool.tile([P, NCOL], f32, name=pfx + "fl")
    nc.vector.tensor_tensor(out=fl[:], in0=xf[:], in1=negm[:], op=_alu("subtract"))
    fr = pool.tile([P, NCOL], f32, name=pfx + "fr")
    nc.vector.tensor_tensor(out=fr[:], in0=x, in1=fl[:], op=_alu("subtract"))
    return fl, fr


def build_program(mm_dtype=f32, fake_gather=False, vol_bf16=False, wbufs=2, vol_dma_eng="sync"):
    """Build the (SPMD, per-core) bass program. All shapes static.

    fake_gather=True (analysis only): replace the indirect gather with a
    static strided DMA of identical descriptor structure (128 descriptors x
    3496 B from the same DRAM tensor) so the cost-model timeline reflects the
    real HW descriptor count. Produces wrong data - never use for results.

    vol_bf16=True: stage the row-dot volume in bf16 (halves the DRAM round
    trip; ~0.4% relative rounding on the pre-combine dots).
    """
    vdt = mybir.dt.bfloat16 if vol_bf16 else f32
    nc = bacc.Bacc()
    P = 128

    # matmul input handling by dtype:
    #  float32  - plain loads, 4 cyc/row
    #  float32r - declare params+tiles as float32r (same bits as f32), 1 cyc/row
    #  bfloat16 - SWDGE casts f32->bf16 during the load, 1 cyc/row
    mdt = mm_dtype
    pdt = mdt if mdt == mybir.dt.float32r else f32
    f1g = nc.declare_dram_parameter("f1g", [C, PPAD], pdt, isOutput=False)
    f2win = nc.declare_dram_parameter("f2win", [C, W18], pdt, isOutput=False)
    coords_t = nc.declare_dram_parameter("coords_t", [PPAD, 4], f32, isOutput=False)
    w_dapT = nc.declare_dram_parameter("w_dapT", [D * D, D * D], f32, isOutput=False)
    out_p = nc.declare_dram_parameter("out", [D * D, PPAD], f32, isOutput=True)

    vols = [nc.dram_tensor(f"vol{t}", [VROWS * STRIDE], vdt) for t in range(NT)]

    with tile.TileContext(nc) as tc:
        with tc.tile_pool(name="const", bufs=1) as cpool, \
             tc.tile_pool(name="work", bufs=wbufs) as wpool, \
             tc.tile_pool(name="mmps", bufs=6, space="PSUM") as mmpool, \
             tc.tile_pool(name="tps", bufs=1, space="PSUM") as tpool, \
             tc.tile_pool(name="ops", bufs=1, space="PSUM") as opool:

            # ---------------- loads ----------------
            mm_cast = (mdt == mybir.dt.bfloat16)
            dma_eng = nc.gpsimd if mm_cast else nc.sync
            f1sb = []
            for k in range(2):
                t_ = cpool.tile([P, PPAD], mdt, name=f"f1sb{k}")
                dma_eng.dma_start(out=t_[:], in_=f1g[k * P:(k + 1) * P, :])
                f1sb.append(t_)
            f2sb = []
            for k in range(2):
                # layout: [0:HEAD)=0, [HEAD:HEAD+W18)=f2 rows, rest 0
                t_ = cpool.tile([P, STRIDE], mdt, name=f"f2sb{k}")
                # memset doesn't accept float32r; zero through a f32 view
                zv0 = t_[:, 0:HEAD]
                zv1 = t_[:, HEAD + W18:STRIDE]
                if mdt == mybir.dt.float32r:
                    zv0, zv1 = zv0.bitcast(f32), zv1.bitcast(f32)
                nc.vector.memset(zv0, 0.0)
                nc.vector.memset(zv1, 0.0)
                dma_eng.dma_start(out=t_[:, HEAD:HEAD + W18], in_=f2win[k * P:(k + 1) * P, :])
                f2sb.append(t_)
            wd = cpool.tile([D * D, D * D], f32, name="wd")
            nc.sync.dma_start(out=wd[:], in_=w_dapT[:, :])
            ident = cpool.tile([P, P], f32, name="ident")
            make_identity(nc, ident[:])
            ct = cpool.tile([P, NT, 4], f32, name="ct")
            nc.sync.dma_start(out=ct[:], in_=coords_t.rearrange("(t l) c -> l t c", l=P))

            # ---------------- per-pixel prep (once) ----------------
            cx = cpool.tile([P, NT], f32, name="cx")
            nc.vector.tensor_copy(out=cx[:], in_=ct[:, :, 0])
            cy = cpool.tile([P, NT], f32, name="cy")
            nc.vector.tensor_copy(out=cy[:], in_=ct[:, :, 1])
            cyadj = cpool.tile([P, NT], f32, name="cyadj")
            nc.vector.tensor_copy(out=cyadj[:], in_=ct[:, :, 2])

            x0f, fx = _floor_frac(nc, cpool, cx[:], P, NT, 'x_')
            y0f, fy = _floor_frac(nc, cpool, cy[:], P, NT, 'y_')
            # floor(cyadj) = cyadj - fy (rowstart integer shift keeps frac)
            y0adjf = cpool.tile([P, NT], f32, name="y0adjf")
            nc.vector.tensor_tensor(out=y0adjf[:], in0=cyadj[:], in1=fy[:],
                                    op=_alu("subtract"))

            xsf = cpool.tile([P, NT], f32, name="xsf")   # x0 - 4
            nc.vector.tensor_scalar(out=xsf[:], in0=x0f[:], scalar1=-4.0,
                                    scalar2=None, op0=_alu("add"))
            ysf = cpool.tile([P, NT], f32, name="ysf")   # y0 - 4 (true rows)
            nc.vector.tensor_scalar(out=ysf[:], in0=y0f[:], scalar1=-4.0,
                                    scalar2=None, op0=_alu("add"))

            # gather offsets: p*STRIDE + (y0adj-4)*96 + (x0-4) + HEAD
            basei = cpool.tile([P, 1], i32, name="basei")
            nc.gpsimd.iota(basei[:], pattern=[[0, 1]], base=HEAD,
                           channel_multiplier=STRIDE)
            basef = cpool.tile([P, 1], f32, name="basef")
            nc.vector.tensor_copy(out=basef[:], in_=basei[:])
            offf = cpool.tile([P, NT], f32, name="offf")
            nc.vector.tensor_scalar(out=offf[:], in0=y0adjf[:], scalar1=96.0,
                                    scalar2=float(-4 * 96 - 4), op0=_alu("mult"),
                                    op1=_alu("add"))
            nc.vector.tensor_tensor(out=offf[:], in0=offf[:], in1=x0f[:], op=_alu("add"))
            nc.vector.tensor_tensor(out=offf[:], in0=offf[:],
                                    in1=basef[:].to_broadcast([P, NT]), op=_alu("add"))
            offi = cpool.tile([P, NT], i32, name="offi")
            nc.vector.tensor_copy(out=offi[:], in_=offf[:])

            # bilinear weights (y weights carry the 1/16 scale)
            wx0 = cpool.tile([P, NT], f32, name="wx0")
            nc.vector.tensor_scalar(out=wx0[:], in0=fx[:], scalar1=-1.0, scalar2=1.0,
                                    op0=_alu("mult"), op1=_alu("add"))
            wy0 = cpool.tile([P, NT], f32, name="wy0")
            nc.vector.tensor_scalar(out=wy0[:], in0=fy[:], scalar1=-0.0625, scalar2=0.0625,
                                    op0=_alu("mult"), op1=_alu("add"))
            wy1 = cpool.tile([P, NT], f32, name="wy1")
            nc.vector.tensor_scalar(out=wy1[:], in0=fy[:], scalar1=0.0625, scalar2=None,
                                    op0=_alu("mult"))

            # iota 0..9 as f32
            iu = cpool.tile([P, 10], i32, name="iu")
            nc.gpsimd.iota(iu[:], pattern=[[1, 10]], base=0, channel_multiplier=0)
            iuf = cpool.tile([P, 10], f32, name="iuf")
            nc.vector.tensor_copy(out=iuf[:], in_=iu[:])

            # validity: vx[p,t,u] = 0 <= x0-4+u <= 95 ; vy similar with 47
            def _valid(base_tile, hi, nm):
                col = cpool.tile([P, NT, 10], f32, name=nm + "c")
                nc.vector.tensor_tensor(
                    out=col[:], in0=base_tile[:].unsqueeze(2).to_broadcast([P, NT, 10]),
                    in1=iuf[:].unsqueeze(1).to_broadcast([P, NT, 10]), op=_alu("add"))
                v0 = cpool.tile([P, NT, 10], f32, name=nm + "0")
                nc.vector.tensor_scalar(out=v0[:], in0=col[:], scalar1=0.0,
                                        scalar2=None, op0=_alu("is_ge"))
                v1 = cpool.tile([P, NT, 10], f32, name=nm + "1")
                nc.vector.tensor_scalar(out=v1[:], in0=col[:], scalar1=float(hi),
                                        scalar2=None, op0=_alu("is_le"))
                v = cpool.tile([P, NT, 10], f32, name=nm)
                nc.vector.tensor_tensor(out=v[:], in0=v0[:], in1=v1[:], op=_alu("mult"))
                return v

            vx = _valid(xsf, W - 1, "vx")
            vy = _valid(ysf, H - 1, "vy")

            # fused weight*validity tables [P, NT, 9]
            def _wtab(v, sl, wsc, nm):
                o = cpool.tile([P, NT, 9], f32, name=nm)
                nc.vector.tensor_tensor(
                    out=o[:], in0=v[:, :, sl],
                    in1=wsc[:].unsqueeze(2).to_broadcast([P, NT, 9]), op=_alu("mult"))
                return o

            X0a = _wtab(vx, slice(0, 9), wx0, "X0a")
            X1a = _wtab(vx, slice(1, 10), fx, "X1a")
            W0a = _wtab(vy, slice(0, 9), wy0, "W0a")
            W1a = _wtab(vy, slice(1, 10), wy1, "W1a")

            outsb_all = cpool.tile([D * D, NT, P], f32, name="outsb_all")

            # ---------------- main loop over M-tiles ----------------
            for t in range(NT):
                # matmul: vol[p, 1792] = f1g_tile^T @ f2win(padded)
                psums = []
                for ci, (n0, w_) in enumerate(CHUNKS):
                    ps = mmpool.tile([P, 512], f32, tag="mmps")
                    for k in range(2):
                        nc.tensor.matmul(out=ps[:, :w_],
                                         lhsT=f1sb[k][:, t * P:(t + 1) * P],
                                         rhs=f2sb[k][:, n0:n0 + w_],
                                         start=(k == 0), stop=(k == 1))
                    psums.append(ps)
                volsb = wpool.tile([P, STRIDE], vdt, tag="volsb")
                for ci, (n0, w_) in enumerate(CHUNKS):
                    eng = nc.vector if ci % 2 == 0 else nc.scalar
                    if eng is nc.scalar:
                        nc.scalar.copy(out=volsb[:, n0:n0 + w_], in_=psums[ci][:, :w_])
                    else:
                        nc.vector.tensor_copy(out=volsb[:, n0:n0 + w_], in_=psums[ci][:, :w_])
                vol2d = vols[t].rearrange("(r s) -> r s", s=STRIDE)
                getattr(nc, vol_dma_eng).dma_start(out=vol2d[:, :], in_=volsb[:])

                # indirect gather: one 874-elem contiguous span per pixel
                g = wpool.tile([P, GAT], vdt, tag="g874")
                if fake_gather:
                    nc.gpsimd.dma_start(out=g[:, :], in_=vol2d[:, :GAT])
                else:
                    # View the vol as [256, 896] rows: a plain reshape, so the
                    # flattened view is identity and element offsets are
                    # unchanged; axis=1 makes the index coefficient 1. The wide
                    # contiguous last dim lets the DGE cover each pixel's 874-
                    # element span with ONE 3496B descriptor instead of 874
                    # 4B descriptors (cost: 1.2us vs 49us per tile).
                    volv = vols[t].rearrange("(n k) -> n k", k=896)
                    nc.gpsimd.indirect_dma_start(
                        out=g[:, :], out_offset=None, in_=volv[:, :],
                        in_offset=bass.IndirectOffsetOnAxis(ap=offi[:, t:t + 1], axis=1))

                # Hx: hx[p, i*10+v] = X0[p,i]*E[p,v,i] + X1[p,i]*E[p,v,i+1]
                #   E[p,v,u] = g[p, v*96+u]
                gap = g[:]
                pdim = gap.ap[0]
                E0 = bass.AP(gap.tensor, gap.offset, [pdim, [1, 9], [W, 10]])
                E1 = bass.AP(gap.tensor, gap.offset + 1, [pdim, [1, 9], [W, 10]])
                hx0 = wpool.tile([P, 90], f32, tag="hx0")
                hx1 = wpool.tile([P, 90], f32, tag="hx1")
                hxs = wpool.tile([P, 90], f32, tag="hxs")
                h3 = lambda x: x[:].rearrange("p (i v) -> p i v", i=9)
                nc.vector.tensor_tensor(
                    out=h3(hx0), in0=E0,
                    in1=X0a[:, t, :].unsqueeze(2).to_broadcast([P, 9, 10]), op=_alu("mult"))
                nc.vector.tensor_tensor(
                    out=h3(hx1), in0=E1,
                    in1=X1a[:, t, :].unsqueeze(2).to_broadcast([P, 9, 10]), op=_alu("mult"))
                nc.vector.tensor_tensor(out=hxs[:], in0=hx0[:], in1=hx1[:], op=_alu("add"))

                # Hy: corr[p, i*9+j] = W0[p,j]*hx[p,i*10+j] + W1[p,j]*hx[p,i*10+j+1]
                hxv = h3(hxs)
                c0 = wpool.tile([P, 81], f32, tag="c0")
                c1 = wpool.tile([P, 81], f32, tag="c1")
                corr = wpool.tile([P, 81], f32, tag="corr")
                c3 = lambda x: x[:].rearrange("p (i j) -> p i j", i=9)
                nc.vector.tensor_tensor(
                    out=c3(c0), in0=hxv[:, :, 0:9],
                    in1=W0a[:, t, :].unsqueeze(1).to_broadcast([P, 9, 9]), op=_alu("mult"))
                nc.vector.tensor_tensor(
                    out=c3(c1), in0=hxv[:, :, 1:10],
                    in1=W1a[:, t, :].unsqueeze(1).to_broadcast([P, 9, 9]), op=_alu("mult"))
                nc.vector.tensor_tensor(out=corr[:], in0=c0[:], in1=c1[:], op=_alu("add"))

                # transpose -> [81, 128], then DAP matmul
                psT = tpool.tile([D * D, P], f32, tag="tps")
                nc.tensor.transpose(out=psT[:], in_=corr[:], identity=ident[:])
                corrT = wpool.tile([D * D, P], f32, tag="corrT")
                nc.scalar.copy(out=corrT[:], in_=psT[:])
                psO = opool.tile([D * D, P], f32, tag="ops")
                nc.tensor.matmul(out=psO[:], lhsT=wd[:], rhs=corrT[:],
                                 start=True, stop=True)
                nc.scalar.copy(out=outsb_all[:, t, :], in_=psO[:])
            nc.sync.dma_start(out=out_p[:, :], in_=outsb_all[:].rearrange("o t p -> o (t p)"))

    nc.compile()
    return nc


def shard_inputs(f1, f2, coords):
    """Bucket pixels by floor(cy), split into 8 equal chunks, build per-core arrays."""
    cx = np.asarray(coords[0, 0].reshape(-1), dtype=np.float32)
    cy = np.asarray(coords[0, 1].reshape(-1), dtype=np.float32)
    y0 = np.floor(cy).astype(np.int64)
    perm = np.argsort(y0, kind="stable")
    f1f = np.asarray(f1[0].reshape(C, HW), dtype=np.float32)
    f2f = np.asarray(f2[0], dtype=np.float32)  # [C, H, W]

    in_maps = []
    for c in range(NCORES):
        sel = perm[c * PIX:(c + 1) * PIX]
        ys = y0[sel]
        y_lo, y_hi = int(ys.min()), int(ys.max())
        assert y_hi - y_lo <= 8, f"core {c}: y0 span {y_hi - y_lo + 1} > 9 unsupported"
        rowstart = y_lo - 4
        rows = np.clip(np.arange(rowstart, rowstart + WINROWS), 0, H - 1)
        f2w = np.ascontiguousarray(f2f[:, rows, :].reshape(C, W18))

        f1gc = np.zeros((C, PPAD), dtype=np.float32)
        f1gc[:, :PIX] = f1f[:, sel]
        ctc = np.zeros((PPAD, 4), dtype=np.float32)
        ctc[:PIX, 0] = cx[sel]
        ctc[:PIX, 1] = cy[sel]
        ctc[:PIX, 2] = cy[sel] - rowstart
        # pad pixels: mid-window coords so all indices stay in range
        ctc[PIX:, 0] = 10.3
        ctc[PIX:, 1] = 10.3
        ctc[PIX:, 2] = 8.3
        in_maps.append({"f1g": f1gc, "f2win": f2w, "coords_t": ctc})
    return in_maps, perm


_PROG_CACHE = {}


def _run(f1, f2, coords, w_dap, mm_dtype_name="bfloat16", trace=False, **build_kw):
    mm_dtype = getattr(mybir.dt, mm_dtype_name)
    key = (mm_dtype_name, tuple(sorted(build_kw.items())))
    if key not in _PROG_CACHE:
        _PROG_CACHE[key] = build_program(mm_dtype=mm_dtype, **build_kw)
    nc = _PROG_CACHE[key]

    in_maps, perm = shard_inputs(f1, f2, coords)
    wdT = np.ascontiguousarray(np.asarray(w_dap, dtype=np.float32).T)
    for m in in_maps:
        m["w_dapT"] = wdT

    res = run_bass_kernel_spmd(nc, in_maps, core_ids=list(range(NCORES)), trace=trace)
    sorted_out = np.concatenate(
        [res.results[c]["out"][:, :PIX] for c in range(NCORES)],
        axis=1)  # [81, 4608] in sorted-pixel order
    full = np.empty((D * D, HW), dtype=np.float32)
    full[:, perm] = sorted_out
    out = full.reshape(1, D * D, H, W)
    return out, res


def kernel(f1, f2, coords, w_dap):
    out, _ = _run(f1, f2, coords, w_dap)
    return out

